# revision 13
# baseline (speedup 1.0000x reference)
"""Trainium2 Bass kernel for nn_BinaryTokenClassificationModel (segment_reduce).

Math: the reference pools token embeddings into word embeddings (mean over
contiguous runs of equal word ids), then computes
    logits[b,s,t] = src_pooled[b,s] @ w_src + tgt_pooled[b,t] @ w_tgt + b.
Because the classifier is linear, pooling and projection commute:
    src_proj[w] = sum_t A[w,t] * (tok_h[t] @ w_src)     (A = 1/count-weighted
    tgt_proj[w] = sum_t A[w,t] * (tok_h[t] @ w_tgt)      segment membership)
and the output is the outer sum src_proj[s] + tgt_proj[t] + b.

Device plan (per core = one batch row, no collectives):
  - ONE SWDGE cast-DMA brings in a host-packed [128, 3072+512] fp32 blob as
    bf16: each partition row carries all 4 token chunks (12 KB descriptors
    amortize the per-descriptor DMA latency) plus the 4 host-precomputed
    membership tiles atw_c[t,w] = (seg[t]==w)*1/count.
  - Weight rows broadcast across partitions with K=1 ones-matmuls on the
    (otherwise idle) TensorEngine into PSUM; ACT copies them to SBUF bf16.
    No GpSimd custom ops -> no ucode library load.
  - Per chunk: DVE native tensor_tensor_reduce computes u[t] = tok[t,:].w
    (bf16 in, fp32 accum); ACT broadcasts u along the free dim into a
    [128,128] bf16 tile; one bf16 matmul per chunk accumulates
    segment-pooling + outer-sum straight into the [S,T] PSUM output.
  - Bias joins as a K=1 matmul; ACT copies PSUM -> SBUF; HWDGE stores out.
"""

import functools

import ml_dtypes
import numpy as np

import concourse.bacc as bacc
import concourse.mybir as mybir
from concourse.bass_utils import run_bass_kernel_spmd
from concourse.tile import TileContext

# Problem geometry (hardcoded per spec)
B = 8
L_SRC = 256
L_TGT = 256
L = L_SRC + L_TGT  # 512
H = 768
P = 128            # SBUF partitions / tokens per chunk
NCHUNK = L // P    # 4
N_SRC_CHUNKS = L_SRC // P  # 2
N_CORES = 8
F32 = mybir.dt.float32
BF16 = mybir.dt.bfloat16

TOK_COLS = NCHUNK * H          # 3072


# ---------------------------------------------------------------------------
# Host-side segment bookkeeping (exact mirror of reference._pool_words)
# ---------------------------------------------------------------------------

def _segments(combined_wid, attention_mask, n_words):
    """Per-token dense run ids exactly as the reference computes them."""
    valid = (attention_mask > 0) & (combined_wid >= 0)  # [B, L]
    prev_wid = np.concatenate(
        [np.full((combined_wid.shape[0], 1), -2, dtype=combined_wid.dtype),
         combined_wid[:, :-1]], axis=1)
    prev_valid = np.concatenate(
        [np.zeros((valid.shape[0], 1), dtype=bool), valid[:, :-1]], axis=1)
    new_run = valid & ((combined_wid != prev_wid) | (~prev_valid))
    run_id = np.cumsum(new_run.astype(np.int64), axis=1) - 1  # [B, L]
    seg = np.where(valid, run_id, n_words)  # n_words = dummy slot
    return seg, valid


def _seg_weights(seg, valid, n_words):
    """1/max(count,1) weight for each token's segment (0 for invalid)."""
    Bv, Lv = seg.shape
    wgt = np.zeros((Bv, Lv), dtype=np.float32)
    for b in range(Bv):
        counts = np.bincount(seg[b][valid[b]], minlength=Lv + 1).astype(np.float32)
        inv = 1.0 / np.maximum(counts, 1.0)
        wgt[b] = np.where(valid[b] & (seg[b] < n_words), inv[np.minimum(seg[b], Lv)], 0.0)
    return wgt


# ---------------------------------------------------------------------------
# Device kernel
# ---------------------------------------------------------------------------

def _emit_body(nc, tc, S, T, aw):
    """aw = atw column width per chunk (S..block / S+T..general layout width).

    blob columns, per half h in {0,1} (chunks 2h, 2h+1):
      [h*hc, h*hc+2H)           the two token chunks
      [h*hc+2H, h*hc+2H+2*aw)   their atw tiles
    Block layout (aw=P): chunk c<2 pools into src cols, c>=2 into tgt cols.
    General layout (aw=S+T): every chunk has both src and tgt atw columns,
    and both u_src and u_tgt are computed per chunk.
    """
    hc = 2 * H + 2 * aw
    ncol = 2 * hc
    blob_d = nc.declare_dram_parameter("blob", [P, ncol], F32, isOutput=False)
    wcat_d = nc.declare_dram_parameter("wcat", [1, 2 * H + 1], BF16, isOutput=False)
    out_d = nc.declare_dram_parameter("out", [S, T], F32, isOutput=True)
    general = aw != P

    CP = mybir.ActivationFunctionType.Copy
    ID = mybir.ActivationFunctionType.Identity

    with (
        tc.tile_pool(name="const", bufs=1) as cpool,
        tc.tile_pool(name="blobp", bufs=1) as bpool,
        tc.tile_pool(name="prods", bufs=2) as ppool,
        tc.tile_pool(name="psum", bufs=1, space="PSUM") as pspool,
    ):
        # token+membership load in two fp32 halves on the HWDGE (sync) ring:
        # no SWDGE wakeup latency, ~28 KB descriptors, first byte ~0.6 us
        # after issue; chunk 0/1 compute hides under half 2's flight
        blob_sb = bpool.tile([P, ncol], F32)
        nc.sync.dma_start(out=blob_sb[:, 0:hc], in_=blob_d[:, 0:hc])
        nc.sync.dma_start(out=blob_sb[:, hc:ncol], in_=blob_d[:, hc:ncol])

        # weights+bias row, already bf16 on host: rides the idle ACT ring
        wcat_bf = cpool.tile([1, 2 * H + 1], BF16)
        nc.scalar.dma_start(out=wcat_bf[:], in_=wcat_d[:])

        ones_bf = cpool.tile([1, P], BF16)
        nc.vector.memset(ones_bf[:], 1.0)

        # broadcast w_src / w_tgt down the partitions: K=1 bf16 matmuls ->
        # PSUM (w_src first -- it gates the first AMR; copies split DVE/ACT)
        wb_ps = []
        for wi in range(2):
            ps = pspool.tile([P, 1024], F32, name=f"wbps_{wi}")
            for j0, j1 in ((0, 512), (512, H)):
                nc.tensor.matmul(
                    ps[:, j0:j1], ones_bf[0:1, 0:P],
                    wcat_bf[0:1, wi * H + j0:wi * H + j1],
                    start=True, stop=True)
            wb_ps.append(ps)
        wb_sb = []
        for wi, eng in ((0, nc.vector), (1, None)):
            wb = cpool.tile([P, H], F32, name=f"wb_{wi}")
            if eng is None:
                nc.scalar.activation(wb[:], wb_ps[wi][:, 0:H], CP)
            else:
                eng.tensor_copy(wb[:], wb_ps[wi][:, 0:H])
            wb_sb.append(wb)

        # bias column b*ones[S,1] via K=1 matmul; used as ACT bias at the end
        bcol_ps = pspool.tile([S, 1], F32)
        nc.tensor.matmul(bcol_ps[:], ones_bf[0:1, 0:S],
                         wcat_bf[0:1, 2 * H:2 * H + 1], start=True, stop=True)
        bcol_sb = cpool.tile([S, 1], F32)
        nc.scalar.activation(bcol_sb[:], bcol_ps[:], CP)

        psum_out = pspool.tile([S, T], F32)
        n_mm = 2 * NCHUNK if general else NCHUNK
        u_sb = cpool.tile([P, 2 * NCHUNK], F32)
        ub_sb = cpool.tile([P, 2 * NCHUNK], BF16)

        mm_i = 0
        for c in range(NCHUNK):
            h, idx = divmod(c, 2)
            tok_c = blob_sb[:, h * hc + idx * H:h * hc + (idx + 1) * H]
            a0 = h * hc + 2 * H + idx * aw
            atw_bf = ppool.tile([P, aw], BF16, name=f"atwb_{c}", tag="atwb")
            nc.gpsimd.tensor_copy(atw_bf[:], blob_sb[:, a0:a0 + aw])
            sides = ((0, True), (1, False)) if general else (
                ((0, True),) if c < N_SRC_CHUNKS else ((1, False),))
            for wi, is_src in sides:
                ucol = u_sb[:, mm_i:mm_i + 1]
                prod = ppool.tile([P, H], F32, name=f"prod_{mm_i}")
                if general or c < NCHUNK - 1:
                    # fused multiply+reduce on DVE
                    nc.vector.affine_mul_reduce(
                        out=prod[:], accum_out=ucol, in0=tok_c,
                        in1=wb_sb[wi][:], scale=1.0, bias=0.0)
                else:
                    # last chunk: Pool does the multiply in parallel with the
                    # previous chunk's AMR; DVE only reduces (~0.4 us shorter)
                    nc.gpsimd.tensor_tensor(out=prod[:], in0=tok_c,
                                            in1=wb_sb[wi][:],
                                            op=mybir.AluOpType.mult)
                    nc.vector.tensor_reduce(out=ucol, in_=prod[:],
                                            axis=mybir.AxisListType.X,
                                            op=mybir.AluOpType.add)
                ubcol = ub_sb[:, mm_i:mm_i + 1]
                nc.gpsimd.tensor_copy(ubcol, ucol)
                first = mm_i == 0
                last = mm_i == n_mm - 1
                if general:
                    a = atw_bf[:, 0:S] if is_src else atw_bf[:, S:S + T]
                else:
                    a = atw_bf[:]
                if is_src:
                    nc.tensor.matmul(psum_out[:], a, ubcol.broadcast_to([P, T]),
                                     start=first, stop=last)
                else:
                    nc.tensor.matmul(psum_out[:], ubcol.broadcast_to([P, S]), a,
                                     start=first, stop=last)
                mm_i += 1

        out_sb = cpool.tile([S, T], F32)
        nc.scalar.activation(out_sb[:], psum_out[:], ID, bias=bcol_sb[0:S, 0:1])
        nc.sync.dma_start(out=out_d[:], in_=out_sb[:])


@functools.lru_cache(maxsize=4)
def _build(S, T, block_ok):
    nc = bacc.Bacc("TRN2", debug=False, num_devices=N_CORES)
    with TileContext(nc) as tc:
        _emit_body(nc, tc, S, T, P if block_ok else S + T)
    nc.compile()
    return nc


# ---------------------------------------------------------------------------
# Host wrapper
# ---------------------------------------------------------------------------

def _prep(inputs):
    tok_h = np.ascontiguousarray(np.asarray(inputs["tok_h"], dtype=np.float32))
    mask = np.asarray(inputs["attention_mask"])
    swid = np.asarray(inputs["source_word_ids"])
    twid = np.asarray(inputs["target_word_ids"])
    W = np.asarray(inputs["W"], dtype=np.float32)
    b = np.asarray(inputs["b"], dtype=np.float32)
    S = int(np.asarray(inputs["S"]))
    T = int(np.asarray(inputs["T"]))

    Bv, Lv, Hv = tok_h.shape
    assert (Bv, Lv, Hv) == (B, L, H), f"unexpected tok_h shape {tok_h.shape}"
    assert swid.shape == (B, L_SRC) and twid.shape == (B, L_TGT)
    assert S <= P and T <= P

    NW = S + T
    combined = np.concatenate([swid, twid], axis=1).astype(np.int64)
    seg, valid = _segments(combined, mask, NW)
    wgt = _seg_weights(seg, valid, NW)

    src_tok_seg = seg[:, :L_SRC][valid[:, :L_SRC]]
    tgt_tok_seg = seg[:, L_SRC:][valid[:, L_SRC:]]
    block_ok = bool(
        (src_tok_seg < S).all()
        and (tgt_tok_seg >= S).all() and (tgt_tok_seg < NW).all()
    )

    wcat = np.zeros((1, 2 * H + 1), dtype=np.float32)
    wcat[0, :H] = W[:H, 0]
    wcat[0, H:2 * H] = W[H:2 * H, 0]
    wcat[0, 2 * H] = b.reshape(-1)[0]
    wcat = wcat.astype(ml_dtypes.bfloat16)

    aw = P if block_ok else NW
    # atw[b, c, p, w] = wgt for the word column this token pools into
    atw = np.zeros((B, NCHUNK, P, aw), dtype=np.float32)
    bi, ti = np.nonzero(valid & (seg < NW))
    sg = seg[bi, ti]
    col = (sg - np.where(sg >= S, S, 0)) if block_ok else sg
    atw[bi, ti // P, ti % P, col] = wgt[bi, ti]

    # per half h: [tok_{2h} | tok_{2h+1} | atw_{2h} | atw_{2h+1}]
    tok4 = tok_h.reshape(B, 2, 2, P, H)         # [B, half, idx, p, H]
    halves = []
    for h in range(2):
        t_cols = tok4[:, h].transpose(0, 2, 1, 3).reshape(B, P, 2 * H)
        a_cols = atw[:, 2 * h:2 * h + 2].transpose(0, 2, 1, 3).reshape(
            B, P, 2 * aw)
        halves.append(np.concatenate([t_cols, a_cols], axis=2))
    blob = np.ascontiguousarray(np.concatenate(halves, axis=2))

    in_maps = []
    for i in range(N_CORES):
        bix = i % B
        in_maps.append({"blob": blob[bix], "wcat": wcat})
    return S, T, block_ok, in_maps


def kernel(**inputs):
    S, T, block_ok, in_maps = _prep(inputs)
    nc = _build(S, T, block_ok)
    res = run_bass_kernel_spmd(nc, in_maps, core_ids=list(range(N_CORES)))
    return np.stack([res.results[i]["out"] for i in range(B)], axis=0)


# revision 15
# speedup vs baseline: 1.2639x; 1.2639x over previous
"""Trainium2 Bass kernel for nn_BinaryTokenClassificationModel (segment_reduce).

Math: the reference pools token embeddings into word embeddings (mean over
contiguous runs of equal word ids), then computes
    logits[b,s,t] = src_pooled[b,s] @ w_src + tgt_pooled[b,t] @ w_tgt + b.
Because the classifier is linear, pooling and projection commute:
    src_proj[w] = sum_t A[w,t] * (tok_h[t] @ w_src)     (A = 1/count-weighted
    tgt_proj[w] = sum_t A[w,t] * (tok_h[t] @ w_tgt)      segment membership)
and the output is the outer sum src_proj[s] + tgt_proj[t] + b.

Device plan (per core = one batch row, no collectives):
  - ONE SWDGE cast-DMA brings in a host-packed [128, 3072+512] fp32 blob as
    bf16: each partition row carries all 4 token chunks (12 KB descriptors
    amortize the per-descriptor DMA latency) plus the 4 host-precomputed
    membership tiles atw_c[t,w] = (seg[t]==w)*1/count.
  - Weight rows broadcast across partitions with K=1 ones-matmuls on the
    (otherwise idle) TensorEngine into PSUM; ACT copies them to SBUF bf16.
    No GpSimd custom ops -> no ucode library load.
  - Per chunk: DVE native tensor_tensor_reduce computes u[t] = tok[t,:].w
    (bf16 in, fp32 accum); ACT broadcasts u along the free dim into a
    [128,128] bf16 tile; one bf16 matmul per chunk accumulates
    segment-pooling + outer-sum straight into the [S,T] PSUM output.
  - Bias joins as a K=1 matmul; ACT copies PSUM -> SBUF; HWDGE stores out.
"""

import functools

import ml_dtypes
import numpy as np

import concourse.bacc as bacc
import concourse.mybir as mybir
from concourse.bass_utils import run_bass_kernel_spmd
from concourse.tile import TileContext

# Problem geometry (hardcoded per spec)
B = 8
L_SRC = 256
L_TGT = 256
L = L_SRC + L_TGT  # 512
H = 768
P = 128            # SBUF partitions / tokens per chunk
NCHUNK = L // P    # 4
N_SRC_CHUNKS = L_SRC // P  # 2
N_CORES = 8
F32 = mybir.dt.float32
BF16 = mybir.dt.bfloat16

TOK_COLS = NCHUNK * H          # 3072


# ---------------------------------------------------------------------------
# Host-side segment bookkeeping (exact mirror of reference._pool_words)
# ---------------------------------------------------------------------------

def _segments(combined_wid, attention_mask, n_words):
    """Per-token dense run ids exactly as the reference computes them."""
    valid = (attention_mask > 0) & (combined_wid >= 0)  # [B, L]
    prev_wid = np.concatenate(
        [np.full((combined_wid.shape[0], 1), -2, dtype=combined_wid.dtype),
         combined_wid[:, :-1]], axis=1)
    prev_valid = np.concatenate(
        [np.zeros((valid.shape[0], 1), dtype=bool), valid[:, :-1]], axis=1)
    new_run = valid & ((combined_wid != prev_wid) | (~prev_valid))
    run_id = np.cumsum(new_run.astype(np.int64), axis=1) - 1  # [B, L]
    seg = np.where(valid, run_id, n_words)  # n_words = dummy slot
    return seg, valid


def _seg_weights(seg, valid, n_words):
    """1/max(count,1) weight for each token's segment (0 for invalid)."""
    Bv, Lv = seg.shape
    wgt = np.zeros((Bv, Lv), dtype=np.float32)
    for b in range(Bv):
        counts = np.bincount(seg[b][valid[b]], minlength=Lv + 1).astype(np.float32)
        inv = 1.0 / np.maximum(counts, 1.0)
        wgt[b] = np.where(valid[b] & (seg[b] < n_words), inv[np.minimum(seg[b], Lv)], 0.0)
    return wgt


# ---------------------------------------------------------------------------
# Device kernel
# ---------------------------------------------------------------------------

def _emit_body(nc, tc, S, T, aw):
    """aw = atw column width per chunk (S..block / S+T..general layout width).

    blob columns, per half h in {0,1} (chunks 2h, 2h+1):
      [h*hc, h*hc+2H)           the two token chunks
      [h*hc+2H, h*hc+2H+2*aw)   their atw tiles
    Block layout (aw=P): chunk c<2 pools into src cols, c>=2 into tgt cols.
    General layout (aw=S+T): every chunk has both src and tgt atw columns,
    and both u_src and u_tgt are computed per chunk.
    """
    # blob (all bf16, host-cast): [tok2|tok3 | tok0|tok1 | atw2|atw3|atw0|atw1]
    # piece 1 = the two tgt chunks' tokens; piece 2 = src tokens + all atw.
    # AMR order 2,3,0,1: the chain starts as soon as piece 1 lands, and the
    # last two matmuls are src-type (stationary atw preloadable).
    ORDER = (2, 3, 0, 1)
    ncol = TOK_COLS + NCHUNK * aw
    p1 = 2 * H                     # piece-1 column count
    blob_d = nc.declare_dram_parameter("blob", [P, ncol], BF16, isOutput=False)
    wcat_d = nc.declare_dram_parameter("wcat", [1, 2 * H + 1], BF16, isOutput=False)
    out_d = nc.declare_dram_parameter("out", [S, T], F32, isOutput=True)
    general = aw != P

    CP = mybir.ActivationFunctionType.Copy

    with (
        tc.tile_pool(name="const", bufs=1) as cpool,
        tc.tile_pool(name="blobp", bufs=1) as bpool,
        tc.tile_pool(name="prods", bufs=2) as ppool,
        tc.tile_pool(name="psum", bufs=1, space="PSUM") as pspool,
    ):
        # token+membership load, two bf16 pieces on the HWDGE (sync) ring
        blob_sb = bpool.tile([P, ncol], BF16)
        nc.sync.dma_start(out=blob_sb[:, 0:p1], in_=blob_d[:, 0:p1])
        nc.sync.dma_start(out=blob_sb[:, p1:ncol], in_=blob_d[:, p1:ncol])

        # weights+bias row, already bf16 on host: rides the idle ACT ring
        wcat_bf = cpool.tile([1, 2 * H + 1], BF16)
        nc.scalar.dma_start(out=wcat_bf[:], in_=wcat_d[:])

        ones_bf = cpool.tile([1, P], BF16)
        nc.vector.memset(ones_bf[:], 1.0)

        # broadcast w_src / w_tgt down the partitions: K=1 bf16 matmuls ->
        # PSUM, ACT copies back to SBUF bf16 (w_tgt first: AMR order is tgt
        # chunks first)
        wb_ps = []
        for wi in (1, 0):
            ps = pspool.tile([P, 1024], F32, name=f"wbps_{wi}")
            for j0, j1 in ((0, 512), (512, H)):
                nc.tensor.matmul(
                    ps[:, j0:j1], ones_bf[0:1, 0:P],
                    wcat_bf[0:1, wi * H + j0:wi * H + j1],
                    start=True, stop=True)
            wb_ps.append(ps)
        wb_ps = wb_ps[::-1]
        wb_sb = []
        for wi in (1, 0):
            wb = cpool.tile([P, H], BF16, name=f"wb_{wi}")
            nc.scalar.activation(wb[:], wb_ps[wi][:, 0:H], CP)
            wb_sb.append(wb)
        wb_sb = wb_sb[::-1]

        # bias column b*ones[S,1] via K=1 matmul -> SBUF; added at the end
        bcol_ps = pspool.tile([S, 1], F32)
        nc.tensor.matmul(bcol_ps[:], ones_bf[0:1, 0:S],
                         wcat_bf[0:1, 2 * H:2 * H + 1], start=True, stop=True)
        bcol_sb = cpool.tile([S, 1], F32)
        nc.scalar.activation(bcol_sb[:], bcol_ps[:], CP)

        psum_out = pspool.tile([S, T], F32)
        n_mm = 2 * NCHUNK if general else NCHUNK
        u_sb = cpool.tile([P, 2 * NCHUNK], F32)
        ub_sb = cpool.tile([P, 2 * NCHUNK], BF16)

        tok_off = {c: oi * H for oi, c in enumerate(ORDER)}
        atw_off = {c: TOK_COLS + oi * aw for oi, c in enumerate(ORDER)}

        mm_i = 0
        for c in ORDER:
            tok_c = blob_sb[:, tok_off[c]:tok_off[c] + H]
            atw_c = blob_sb[:, atw_off[c]:atw_off[c] + aw]
            sides = ((0, True), (1, False)) if general else (
                ((0, True),) if c < N_SRC_CHUNKS else ((1, False),))
            for wi, is_src in sides:
                ucol = u_sb[:, mm_i:mm_i + 1]
                prod = ppool.tile([P, H], BF16, name=f"prod_{mm_i}")
                nc.vector.affine_mul_reduce(
                    out=prod[:], accum_out=ucol, in0=tok_c, in1=wb_sb[wi][:],
                    scale=1.0, bias=0.0)
                ubcol = ub_sb[:, mm_i:mm_i + 1]
                nc.vector.tensor_copy(ubcol, ucol)
                first = mm_i == 0
                last = mm_i == n_mm - 1
                if general:
                    a = atw_c[:, 0:S] if is_src else atw_c[:, S:S + T]
                else:
                    a = atw_c
                if is_src:
                    nc.tensor.matmul(psum_out[:], a, ubcol.broadcast_to([P, T]),
                                     start=first, stop=last)
                else:
                    nc.tensor.matmul(psum_out[:], ubcol.broadcast_to([P, S]), a,
                                     start=first, stop=last)
                mm_i += 1

        out_sb = cpool.tile([S, T], F32)
        nc.vector.tensor_scalar_add(out_sb[:], psum_out[:], bcol_sb[0:S, 0:1])
        nc.sync.dma_start(out=out_d[:], in_=out_sb[:])


@functools.lru_cache(maxsize=4)
def _build(S, T, block_ok):
    nc = bacc.Bacc("TRN2", debug=False, num_devices=N_CORES)
    with TileContext(nc) as tc:
        _emit_body(nc, tc, S, T, P if block_ok else S + T)
    nc.compile()
    return nc


# ---------------------------------------------------------------------------
# Host wrapper
# ---------------------------------------------------------------------------

def _prep(inputs):
    tok_h = np.ascontiguousarray(np.asarray(inputs["tok_h"], dtype=np.float32))
    mask = np.asarray(inputs["attention_mask"])
    swid = np.asarray(inputs["source_word_ids"])
    twid = np.asarray(inputs["target_word_ids"])
    W = np.asarray(inputs["W"], dtype=np.float32)
    b = np.asarray(inputs["b"], dtype=np.float32)
    S = int(np.asarray(inputs["S"]))
    T = int(np.asarray(inputs["T"]))

    Bv, Lv, Hv = tok_h.shape
    assert (Bv, Lv, Hv) == (B, L, H), f"unexpected tok_h shape {tok_h.shape}"
    assert swid.shape == (B, L_SRC) and twid.shape == (B, L_TGT)
    assert S <= P and T <= P

    NW = S + T
    combined = np.concatenate([swid, twid], axis=1).astype(np.int64)
    seg, valid = _segments(combined, mask, NW)
    wgt = _seg_weights(seg, valid, NW)

    src_tok_seg = seg[:, :L_SRC][valid[:, :L_SRC]]
    tgt_tok_seg = seg[:, L_SRC:][valid[:, L_SRC:]]
    block_ok = bool(
        (src_tok_seg < S).all()
        and (tgt_tok_seg >= S).all() and (tgt_tok_seg < NW).all()
    )

    wcat = np.zeros((1, 2 * H + 1), dtype=np.float32)
    wcat[0, :H] = W[:H, 0]
    wcat[0, H:2 * H] = W[H:2 * H, 0]
    wcat[0, 2 * H] = b.reshape(-1)[0]
    wcat = wcat.astype(ml_dtypes.bfloat16)

    aw = P if block_ok else NW
    # atw[b, c, p, w] = wgt for the word column this token pools into
    atw = np.zeros((B, NCHUNK, P, aw), dtype=np.float32)
    bi, ti = np.nonzero(valid & (seg < NW))
    sg = seg[bi, ti]
    col = (sg - np.where(sg >= S, S, 0)) if block_ok else sg
    atw[bi, ti // P, ti % P, col] = wgt[bi, ti]

    # column layout (all bf16): [tok_2|tok_3|tok_0|tok_1 | atw_2|atw_3|atw_0|atw_1]
    order = (2, 3, 0, 1)
    tok4 = tok_h.reshape(B, NCHUNK, P, H)
    t_cols = tok4[:, order].transpose(0, 2, 1, 3).reshape(B, P, TOK_COLS)
    a_cols = atw[:, order].transpose(0, 2, 1, 3).reshape(B, P, NCHUNK * aw)
    blob = np.ascontiguousarray(
        np.concatenate([t_cols, a_cols], axis=2).astype(ml_dtypes.bfloat16))

    in_maps = []
    for i in range(N_CORES):
        bix = i % B
        in_maps.append({"blob": blob[bix], "wcat": wcat})
    return S, T, block_ok, in_maps


def kernel(**inputs):
    S, T, block_ok, in_maps = _prep(inputs)
    nc = _build(S, T, block_ok)
    res = run_bass_kernel_spmd(nc, in_maps, core_ids=list(range(N_CORES)))
    return np.stack([res.results[i]["out"] for i in range(B)], axis=0)


# revision 16
# speedup vs baseline: 1.2774x; 1.0107x over previous
"""Trainium2 Bass kernel for nn_BinaryTokenClassificationModel (segment_reduce).

Math: the reference pools token embeddings into word embeddings (mean over
contiguous runs of equal word ids), then computes
    logits[b,s,t] = src_pooled[b,s] @ w_src + tgt_pooled[b,t] @ w_tgt + b.
Because the classifier is linear, pooling and projection commute:
    src_proj[w] = sum_t A[w,t] * (tok_h[t] @ w_src)     (A = 1/count-weighted
    tgt_proj[w] = sum_t A[w,t] * (tok_h[t] @ w_tgt)      segment membership)
and the output is the outer sum src_proj[s] + tgt_proj[t] + b.

Device plan (per core = one batch row, no collectives):
  - ONE SWDGE cast-DMA brings in a host-packed [128, 3072+512] fp32 blob as
    bf16: each partition row carries all 4 token chunks (12 KB descriptors
    amortize the per-descriptor DMA latency) plus the 4 host-precomputed
    membership tiles atw_c[t,w] = (seg[t]==w)*1/count.
  - Weight rows broadcast across partitions with K=1 ones-matmuls on the
    (otherwise idle) TensorEngine into PSUM; ACT copies them to SBUF bf16.
    No GpSimd custom ops -> no ucode library load.
  - Per chunk: DVE native tensor_tensor_reduce computes u[t] = tok[t,:].w
    (bf16 in, fp32 accum); ACT broadcasts u along the free dim into a
    [128,128] bf16 tile; one bf16 matmul per chunk accumulates
    segment-pooling + outer-sum straight into the [S,T] PSUM output.
  - Bias joins as a K=1 matmul; ACT copies PSUM -> SBUF; HWDGE stores out.
"""

import functools

import ml_dtypes
import numpy as np

import concourse.bacc as bacc
import concourse.mybir as mybir
from concourse.bass_utils import run_bass_kernel_spmd
from concourse.tile import TileContext

# Problem geometry (hardcoded per spec)
B = 8
L_SRC = 256
L_TGT = 256
L = L_SRC + L_TGT  # 512
H = 768
P = 128            # SBUF partitions / tokens per chunk
NCHUNK = L // P    # 4
N_SRC_CHUNKS = L_SRC // P  # 2
N_CORES = 8
F32 = mybir.dt.float32
BF16 = mybir.dt.bfloat16

TOK_COLS = NCHUNK * H          # 3072


# ---------------------------------------------------------------------------
# Host-side segment bookkeeping (exact mirror of reference._pool_words)
# ---------------------------------------------------------------------------

def _segments(combined_wid, attention_mask, n_words):
    """Per-token dense run ids exactly as the reference computes them."""
    valid = (attention_mask > 0) & (combined_wid >= 0)  # [B, L]
    prev_wid = np.concatenate(
        [np.full((combined_wid.shape[0], 1), -2, dtype=combined_wid.dtype),
         combined_wid[:, :-1]], axis=1)
    prev_valid = np.concatenate(
        [np.zeros((valid.shape[0], 1), dtype=bool), valid[:, :-1]], axis=1)
    new_run = valid & ((combined_wid != prev_wid) | (~prev_valid))
    run_id = np.cumsum(new_run.astype(np.int64), axis=1) - 1  # [B, L]
    seg = np.where(valid, run_id, n_words)  # n_words = dummy slot
    return seg, valid


def _seg_weights(seg, valid, n_words):
    """1/max(count,1) weight for each token's segment (0 for invalid)."""
    Bv, Lv = seg.shape
    wgt = np.zeros((Bv, Lv), dtype=np.float32)
    for b in range(Bv):
        counts = np.bincount(seg[b][valid[b]], minlength=Lv + 1).astype(np.float32)
        inv = 1.0 / np.maximum(counts, 1.0)
        wgt[b] = np.where(valid[b] & (seg[b] < n_words), inv[np.minimum(seg[b], Lv)], 0.0)
    return wgt


# ---------------------------------------------------------------------------
# Device kernel
# ---------------------------------------------------------------------------

def _emit_body(nc, tc, S, T, aw):
    """aw = atw column width per chunk (S..block / S+T..general layout width).

    blob columns, per half h in {0,1} (chunks 2h, 2h+1):
      [h*hc, h*hc+2H)           the two token chunks
      [h*hc+2H, h*hc+2H+2*aw)   their atw tiles
    Block layout (aw=P): chunk c<2 pools into src cols, c>=2 into tgt cols.
    General layout (aw=S+T): every chunk has both src and tgt atw columns,
    and both u_src and u_tgt are computed per chunk.
    """
    # blob (all bf16, host-cast): [tok2|tok3 | tok0|tok1 | atw2|atw3|atw0|atw1]
    # piece 1 = the two tgt chunks' tokens; piece 2 = src tokens + all atw.
    # AMR order 2,3,0,1: the chain starts as soon as piece 1 lands, and the
    # last two matmuls are src-type (stationary atw preloadable).
    ORDER = (2, 3, 0, 1)
    ncol = TOK_COLS + NCHUNK * aw
    p1 = 2 * H                     # piece-1 column count
    blob_d = nc.declare_dram_parameter("blob", [P, ncol], BF16, isOutput=False)
    wcat_d = nc.declare_dram_parameter("wcat", [1, 2 * H + 1], BF16, isOutput=False)
    out_d = nc.declare_dram_parameter("out", [S, T], F32, isOutput=True)
    general = aw != P

    CP = mybir.ActivationFunctionType.Copy

    with (
        tc.tile_pool(name="const", bufs=1) as cpool,
        tc.tile_pool(name="blobp", bufs=1) as bpool,
        tc.tile_pool(name="prods", bufs=2) as ppool,
        tc.tile_pool(name="psum", bufs=1, space="PSUM") as pspool,
    ):
        # token+membership load, two bf16 pieces on the HWDGE (sync) ring
        blob_sb = bpool.tile([P, ncol], BF16)
        nc.sync.dma_start(out=blob_sb[:, 0:p1], in_=blob_d[:, 0:p1])
        nc.sync.dma_start(out=blob_sb[:, p1:ncol], in_=blob_d[:, p1:ncol])

        # weights+bias row, already bf16 on host: rides the idle ACT ring
        wcat_bf = cpool.tile([1, 2 * H + 1], BF16)
        nc.scalar.dma_start(out=wcat_bf[:], in_=wcat_d[:])

        ones_bf = cpool.tile([1, P], BF16)
        nc.vector.memset(ones_bf[:], 1.0)

        # broadcast w_src / w_tgt down the partitions: K=1 bf16 matmuls ->
        # PSUM, ACT copies back to SBUF bf16 (w_tgt first: AMR order is tgt
        # chunks first)
        wb_ps = []
        for wi in (1, 0):
            ps = pspool.tile([P, 1024], F32, name=f"wbps_{wi}")
            for j0, j1 in ((0, 512), (512, H)):
                nc.tensor.matmul(
                    ps[:, j0:j1], ones_bf[0:1, 0:P],
                    wcat_bf[0:1, wi * H + j0:wi * H + j1],
                    start=True, stop=True)
            wb_ps.append(ps)
        wb_ps = wb_ps[::-1]
        wb_sb = []
        for wi in (1, 0):
            wb = cpool.tile([P, H], BF16, name=f"wb_{wi}")
            nc.scalar.activation(wb[:], wb_ps[wi][:, 0:H], CP)
            wb_sb.append(wb)
        wb_sb = wb_sb[::-1]

        # bias column b*ones[S,1] via K=1 matmul -> SBUF; added at the end
        bcol_ps = pspool.tile([S, 1], F32)
        nc.tensor.matmul(bcol_ps[:], ones_bf[0:1, 0:S],
                         wcat_bf[0:1, 2 * H:2 * H + 1], start=True, stop=True)
        bcol_sb = cpool.tile([S, 1], F32)
        nc.scalar.activation(bcol_sb[:], bcol_ps[:], CP)

        psum_out = pspool.tile([S, T], F32)
        n_mm = 2 * NCHUNK if general else NCHUNK
        u_sb = cpool.tile([P, 2 * NCHUNK], F32)
        ub_sb = cpool.tile([P, 2 * NCHUNK], BF16)

        tok_off = {c: oi * H for oi, c in enumerate(ORDER)}
        atw_off = {c: TOK_COLS + oi * aw for oi, c in enumerate(ORDER)}

        mm_i = 0
        for c in ORDER:
            tok_c = blob_sb[:, tok_off[c]:tok_off[c] + H]
            atw_c = blob_sb[:, atw_off[c]:atw_off[c] + aw]
            sides = ((0, True), (1, False)) if general else (
                ((0, True),) if c < N_SRC_CHUNKS else ((1, False),))
            for wi, is_src in sides:
                prod = ppool.tile([P, H], BF16, name=f"prod_{mm_i}")
                ubcol = ub_sb[:, mm_i:mm_i + 1]
                # bf16 accum write == fp32 accum + bf16 cast (the matmul
                # consumes u in bf16 either way); skips a DVE cast per chunk
                with nc.allow_low_precision("u is consumed in bf16 by matmul"):
                    nc.vector.affine_mul_reduce(
                        out=prod[:], accum_out=ubcol, in0=tok_c,
                        in1=wb_sb[wi][:], scale=1.0, bias=0.0)
                first = mm_i == 0
                last = mm_i == n_mm - 1
                if general:
                    a = atw_c[:, 0:S] if is_src else atw_c[:, S:S + T]
                else:
                    a = atw_c
                if is_src:
                    nc.tensor.matmul(psum_out[:], a, ubcol.broadcast_to([P, T]),
                                     start=first, stop=last)
                else:
                    nc.tensor.matmul(psum_out[:], ubcol.broadcast_to([P, S]), a,
                                     start=first, stop=last)
                mm_i += 1

        out_sb = cpool.tile([S, T], F32)
        nc.vector.tensor_scalar_add(out_sb[:], psum_out[:], bcol_sb[0:S, 0:1])
        nc.sync.dma_start(out=out_d[:], in_=out_sb[:])


@functools.lru_cache(maxsize=4)
def _build(S, T, block_ok):
    nc = bacc.Bacc("TRN2", debug=False, num_devices=N_CORES)
    with TileContext(nc) as tc:
        _emit_body(nc, tc, S, T, P if block_ok else S + T)
    nc.compile()
    return nc


# ---------------------------------------------------------------------------
# Host wrapper
# ---------------------------------------------------------------------------

def _prep(inputs):
    tok_h = np.ascontiguousarray(np.asarray(inputs["tok_h"], dtype=np.float32))
    mask = np.asarray(inputs["attention_mask"])
    swid = np.asarray(inputs["source_word_ids"])
    twid = np.asarray(inputs["target_word_ids"])
    W = np.asarray(inputs["W"], dtype=np.float32)
    b = np.asarray(inputs["b"], dtype=np.float32)
    S = int(np.asarray(inputs["S"]))
    T = int(np.asarray(inputs["T"]))

    Bv, Lv, Hv = tok_h.shape
    assert (Bv, Lv, Hv) == (B, L, H), f"unexpected tok_h shape {tok_h.shape}"
    assert swid.shape == (B, L_SRC) and twid.shape == (B, L_TGT)
    assert S <= P and T <= P

    NW = S + T
    combined = np.concatenate([swid, twid], axis=1).astype(np.int64)
    seg, valid = _segments(combined, mask, NW)
    wgt = _seg_weights(seg, valid, NW)

    src_tok_seg = seg[:, :L_SRC][valid[:, :L_SRC]]
    tgt_tok_seg = seg[:, L_SRC:][valid[:, L_SRC:]]
    block_ok = bool(
        (src_tok_seg < S).all()
        and (tgt_tok_seg >= S).all() and (tgt_tok_seg < NW).all()
    )

    wcat = np.zeros((1, 2 * H + 1), dtype=np.float32)
    wcat[0, :H] = W[:H, 0]
    wcat[0, H:2 * H] = W[H:2 * H, 0]
    wcat[0, 2 * H] = b.reshape(-1)[0]
    wcat = wcat.astype(ml_dtypes.bfloat16)

    aw = P if block_ok else NW
    # atw[b, c, p, w] = wgt for the word column this token pools into
    atw = np.zeros((B, NCHUNK, P, aw), dtype=np.float32)
    bi, ti = np.nonzero(valid & (seg < NW))
    sg = seg[bi, ti]
    col = (sg - np.where(sg >= S, S, 0)) if block_ok else sg
    atw[bi, ti // P, ti % P, col] = wgt[bi, ti]

    # column layout (all bf16): [tok_2|tok_3|tok_0|tok_1 | atw_2|atw_3|atw_0|atw_1]
    order = (2, 3, 0, 1)
    tok4 = tok_h.reshape(B, NCHUNK, P, H)
    t_cols = tok4[:, order].transpose(0, 2, 1, 3).reshape(B, P, TOK_COLS)
    a_cols = atw[:, order].transpose(0, 2, 1, 3).reshape(B, P, NCHUNK * aw)
    blob = np.ascontiguousarray(
        np.concatenate([t_cols, a_cols], axis=2).astype(ml_dtypes.bfloat16))

    in_maps = []
    for i in range(N_CORES):
        bix = i % B
        in_maps.append({"blob": blob[bix], "wcat": wcat})
    return S, T, block_ok, in_maps


def kernel(**inputs):
    S, T, block_ok, in_maps = _prep(inputs)
    nc = _build(S, T, block_ok)
    res = run_bass_kernel_spmd(nc, in_maps, core_ids=list(range(N_CORES)))
    return np.stack([res.results[i]["out"] for i in range(B)], axis=0)
